# revision 37
# baseline (speedup 1.0000x reference)
"""Multi-head attention (B=8, N=1024, D=512, H=8) on 8 TRN2 NeuronCores.

Sharding: pure batch-parallel - core i computes batch i end-to-end, no
collectives. Host-side prep per batch: gather valid keys (mask) into a
contiguous buffer padded to NKV=640, pre-transpose x, convert streams to
bf16, and pack every stream so it loads with ONE dma_start (issue on the
shared HWDGE costs ~630ns per instruction and a single queue streams at
~150-200GB/s, so the loads are spread across the SP and ACT HWDGE queues
plus the gpsimd SWDGE queue, with the x streams split in halves).

Device pipeline (bf16 matmuls, f32 PSUM accumulation):
  k^T/q^T/v projections -> per head pair, scores s^T[k,q] land in one
  [128,1024] PSUM tile (head A cols 0:512 on PE row tile 0, head B cols
  512:1024 on row tile 64, overlapping on disjoint PE rows) -> one exp on
  ACT per query-half with the key-padding mask folded into the activation
  bias -> attn@v with an augmented ones-column producing the softmax
  denominator in row 64; the four (head, half) accumulators of a pair
  ring through a 4-buffer PSUM pool so the next pair's AV starts while
  this pair's normalize chains (denominator copy, fast reciprocal, gpsimd
  partition-broadcast, multiply fused with the PSUM->SBUF copy) drain ->
  out-projection. The tail is restructured so the last pair's chain
  latency is covered: out-proj columns for ic 4..7 pre-accumulate their
  dp0-2 terms into retired score-pool PSUM slots, ic 0..3 run from the
  retiring AV slots, and only 4 matmuls + bias + store remain after the
  final chain. y is stored bf16 and upcast on the host.

Math shortcuts: bk is dropped (constant-in-key terms cancel in softmax);
bv is folded into the output bias on the host (bob' = bo + bv @ wo since
normalized attention rows sum to 1). fp8 attn@v was tried and rejected:
v quantization alone puts the max-abs tail at 2.5e-2 (tol 2e-2) and exp
overflows e4m3. Interleaving the projections into the attention phase
was tried and rejected: the PE row rate stays ~0.83ns/row regardless, so
it only moves work between phases, while shrinking the AV PSUM pool to
make room serializes the normalize chains into the pipeline.
"""

import sys

import numpy as np

sys.path.insert(0, "/opt/trn_rl_repo")

B, N, D, H = 8, 1024, 512, 8
HD = D // H            # 64
SCALE = HD ** -0.5     # 0.125
NKV = 640              # padded valid-key count (5 chunks of 128)
KC = NKV // 128        # 5
DC = D // 128          # 4
VW = HD + 2            # 66: aug head stride, 4B-aligned for bf16 weights
PAD_BIAS = -30000.0    # exp(PAD_BIAS + s*SCALE) == 0.0 exactly

_prog_cache = {}


def _build_program():
    import concourse.bacc as bacc
    import concourse.tile as tile
    from concourse import mybir

    dt = mybir.dt
    f32 = dt.float32
    bf16 = dt.bfloat16
    AF = mybir.ActivationFunctionType

    nc = bacc.Bacc("TRN2", target_bir_lowering=False, debug=False)

    xT_d = nc.dram_tensor("xT", [128, DC, N], bf16, kind="ExternalInput").ap()
    xkT_d = nc.dram_tensor("xkT", [128, DC, NKV], bf16,
                           kind="ExternalInput").ap()
    wq_d = nc.dram_tensor("wq", [128, DC, D], bf16, kind="ExternalInput").ap()
    wk_d = nc.dram_tensor("wk", [128, DC, D], bf16, kind="ExternalInput").ap()
    wv_d = nc.dram_tensor("wv", [128, DC, D], bf16, kind="ExternalInput").ap()
    wo_d = nc.dram_tensor("wo", [128, DC, D], bf16, kind="ExternalInput").ap()
    tbl_d = nc.dram_tensor("tbl", [128, DC + KC], f32,
                           kind="ExternalInput").ap()
    bob_d = nc.dram_tensor("bob", [128, D], f32, kind="ExternalInput").ap()
    y_d = nc.dram_tensor("y", [N, D], bf16, kind="ExternalOutput").ap()

    with tile.TileContext(nc) as tc, \
         nc.allow_low_precision(reason="bf16 matmul streams, f32 accum"):
        with tc.tile_pool(name="const", bufs=1) as cpool:
            wk_t = cpool.tile([128, DC, D], bf16, name="wk_t")
            wq_t = cpool.tile([128, DC, D], bf16, name="wq_t")
            wv_t = cpool.tile([128, DC, D], bf16, name="wv_t")
            wo_t = cpool.tile([128, DC, D], bf16, name="wo_t")
            xkT_t = cpool.tile([128, DC, NKV], bf16, name="xkT_t")
            xT_t = cpool.tile([128, DC, N], bf16, name="xT_t")
            kT_t = cpool.tile([128, DC, NKV], bf16, name="kT_t")
            qT_t = cpool.tile([128, DC, N], bf16, name="qT_t")
            vaug_t = [cpool.tile([128, H, VW], bf16, name=f"vaug_t{c}")
                      for c in range(KC)]
            aoT_t = cpool.tile([128, DC, N], bf16, name="aoT_t")
            tbl_t = cpool.tile([128, DC + KC], f32, name="tbl_t")
            bob_t = cpool.tile([128, D], f32, name="bob_t")

            # kproj's inputs split across both HWDGE queues so all four
            # halves land concurrently (~150GB/s per queue); qproj's xT
            # second half rides the SWDGE queue which is otherwise idle
            # until wv is needed.
            nc.sync.dma_start(xkT_t[:, 0:2, :], xkT_d[:, 0:2, :])
            nc.sync.dma_start(wk_t[:, 0:2, :], wk_d[:, 0:2, :])
            nc.sync.dma_start(xT_t[:, 0:2, :], xT_d[:, 0:2, :])
            nc.scalar.dma_start(tbl_t[:], tbl_d[:])
            nc.scalar.dma_start(xkT_t[:, 2:4, :], xkT_d[:, 2:4, :])
            nc.scalar.dma_start(wk_t[:, 2:4, :], wk_d[:, 2:4, :])
            nc.scalar.dma_start(wq_t[:], wq_d[:])
            nc.gpsimd.dma_start(xT_t[:, 2:4, :], xT_d[:, 2:4, :])
            nc.gpsimd.dma_start(wv_t[:], wv_d[:])
            nc.gpsimd.dma_start(bob_t[:], bob_d[:])
            nc.gpsimd.dma_start(wo_t[:], wo_d[:])

            # ones column of the augmented v (denominator trick), plus a
            # dummy exp to pull the ACT table load off the first real exp
            warm_t = cpool.tile([1, 1], f32, name="warm_t")
            nc.vector.memset(warm_t[:], 0.0)
            nc.scalar.activation(warm_t[:], warm_t[:],
                                 mybir.ActivationFunctionType.Exp)
            for c in range(KC):
                nc.vector.memset(vaug_t[c][:, :, HD:HD + 1], 1.0)

            # ---- Phase 1a: k projection (no bias: cancels in softmax) ----
            with tc.tile_pool(name="kpp", bufs=3, space="PSUM") as kpp:
                for dp in range(DC):
                    ps = kpp.tile([128, NKV], f32, name="kps")
                    for dc in range(DC):
                        lhs = wk_t[:, dc, 128 * dp:128 * (dp + 1)]
                        nc.tensor.matmul(
                            ps[:, 0:512], lhs, xkT_t[:, dc, 0:512],
                            start=(dc == 0), stop=(dc == DC - 1),
                        )
                        nc.tensor.matmul(
                            ps[:, 512:NKV], lhs, xkT_t[:, dc, 512:NKV],
                            start=(dc == 0), stop=(dc == DC - 1),
                        )
                    nc.vector.tensor_scalar_add(kT_t[:, dp, :], ps[:], 0.0)

            # ---- Phase 1b: q projection ----
            with tc.tile_pool(name="qpp", bufs=3, space="PSUM") as qpp:
                for dp in range(DC):
                    ps = qpp.tile([128, N], f32, name="qps")
                    for dc in range(DC):
                        lhs = wq_t[:, dc, 128 * dp:128 * (dp + 1)]
                        for hf in range(2):
                            nc.tensor.matmul(
                                ps[:, 512 * hf:512 * (hf + 1)],
                                lhs,
                                xT_t[:, dc, 512 * hf:512 * (hf + 1)],
                                start=(dc == 0), stop=(dc == DC - 1),
                            )
                    nc.vector.tensor_scalar_add(qT_t[:, dp, :], ps[:],
                                                tbl_t[:, dp:dp + 1])

            # ---- Phase 1c: v projection (no bias: folded into bob') ----
            with tc.tile_pool(name="vpp", bufs=2, space="PSUM") as vpp:
                for c in range(KC):
                    ps = vpp.tile([128, H, HD], f32, name="vps")
                    for dc in range(DC):
                        nc.tensor.matmul(
                            ps[:], xkT_t[:, dc, 128 * c:128 * (c + 1)],
                            wv_t[:, dc, :],
                            start=(dc == 0), stop=(dc == DC - 1),
                        )
                    nc.vector.tensor_scalar_add(vaug_t[c][:, :, 0:HD],
                                                ps[:], 0.0)

            # ---- Phase 2: attention on head pairs + restructured tail ----
            with tc.tile_pool(name="scp", bufs=2, space="PSUM") as scp, \
                 tc.tile_pool(name="oap", bufs=4, space="PSUM") as oap, \
                 tc.tile_pool(name="pp", bufs=8) as pp, \
                 tc.tile_pool(name="rcp", bufs=6) as rcp, \
                 tc.tile_pool(name="rbp", bufs=4) as rbp, \
                 tc.tile_pool(name="ysp", bufs=4) as ysp:
                # Software-pipelined across head-pair boundaries: scores
                # run two chunks ahead of attn@v GLOBALLY, so the ACT exp
                # stream never drains while a new pair's pipeline refills.
                oa_t = {}   # (dp, hf, hi) -> [65, 512] PSUM accumulator
                p_t = {}    # (dp, c, hf)  -> [128, N] bf16 exp tile

                def sc_unit(dp, c):
                    for hf in range(2):
                        sc = scp.tile([128, N], f32, name="sc", tag="sc")
                        for hi in range(2):
                            row = HD * hi
                            nc.tensor.matmul(
                                sc[:, 512 * hi:512 * (hi + 1)],
                                kT_t[row:row + HD, dp,
                                     128 * c:128 * (c + 1)],
                                qT_t[row:row + HD, dp,
                                     512 * hf:512 * (hf + 1)],
                                start=True, stop=True,
                            )
                        p = pp.tile([128, N], bf16, name="p")
                        nc.scalar.activation(
                            p[:], sc[:], AF.Exp,
                            bias=tbl_t[:, DC + c:DC + c + 1], scale=SCALE,
                        )
                        p_t[(dp, c, hf)] = p

                def av_unit(dp, c):
                    for hf in range(2):
                        for hi in range(2):
                            if c == 0:
                                oa_t[(dp, hf, hi)] = oap.tile(
                                    [HD + 1, 512], f32, name="oa", tag="oa")
                            nc.tensor.matmul(
                                oa_t[(dp, hf, hi)][:],
                                vaug_t[c][:, 2 * dp + hi, 0:HD + 1],
                                p_t[(dp, c, hf)][:, 512 * hi:512 * (hi + 1)],
                                start=(c == 0), stop=(c == KC - 1),
                            )

                def chains(dp, db_on_act=False):
                    for hf in range(2):
                        for hi in range(2):
                            oa = oa_t[(dp, hf, hi)]
                            row = HD * hi
                            # custom DVE ops read garbage from PSUM on HW:
                            # stage the denominator row through SBUF first
                            # (on ACT for the last pair - it idles there
                            # while DVE still runs the normalize muls)
                            rc = rcp.tile([1, 512], f32, name="rc")
                            nc.vector.reciprocal(rc[:], oa[HD:HD + 1, :])
                            rbs = rbp.tile([HD, 512], f32, name="rbs")
                            nc.gpsimd.partition_broadcast(rbs[:], rc[:])
                            nc.vector.tensor_mul(
                                aoT_t[row:row + HD, dp,
                                      512 * hf:512 * (hf + 1)],
                                oa[0:HD, :], rbs[:])

                seq = [(dp, c) for dp in range(DC) for c in range(KC)]
                for i, (dp, c) in enumerate(seq):
                    sc_unit(dp, c)
                    if i >= 2:
                        adp, ac = seq[i - 2]
                        av_unit(adp, ac)
                        if ac == KC - 1:
                            chains(adp)
                for j in (len(seq) - 2, len(seq) - 1):
                    adp, ac = seq[j]
                    av_unit(adp, ac)
                    if ac == KC - 1:
                        chains(adp, db_on_act=True)

                # ---- restructured out-projection tail ----
                # ic 4..7: pre-accumulate dp0-2 into retired score slots
                # while the dp3 chains drain (their aoT needs only chains
                # of dp0-2, done long ago, plus dp3 which lands last).
                yab = []
                for j in range(2):
                    t = scp.tile([128, 2, 512], f32, name="yab", tag="sc")
                    for icp in range(2):
                        ic = 4 + 2 * j + icp
                        for dp in range(3):
                            nc.tensor.matmul(
                                t[:, icp, :],
                                aoT_t[:, dp, 128 * ic:128 * (ic + 1)],
                                wo_t[:, dp, :], start=(dp == 0), stop=False)
                    yab.append(t)
                # ic 0..3 (full accumulation from the retiring AV slots)
                # interleaved with the ic 4..7 dp3-term finishers so no
                # single consumer chain serializes the stream.
                def op_full(ic):
                    yps = oap.tile([128, 512], f32, name="yps", tag="oa")
                    for dp in range(DC):
                        nc.tensor.matmul(
                            yps[:], aoT_t[:, dp, 128 * ic:128 * (ic + 1)],
                            wo_t[:, dp, :],
                            start=(dp == 0), stop=(dp == DC - 1),
                        )
                    ysb = ysp.tile([128, D], bf16, name="ysb")
                    nc.vector.tensor_add(ysb[:], yps[:], bob_t[:])
                    nc.sync.dma_start(y_d[128 * ic:128 * (ic + 1), :], ysb[:])

                def op_fin(ic):
                    j, icp = divmod(ic - 4, 2)
                    nc.tensor.matmul(
                        yab[j][:, icp, :],
                        aoT_t[:, 3, 128 * ic:128 * (ic + 1)],
                        wo_t[:, 3, :], start=False, stop=True)
                    ysb = ysp.tile([128, D], bf16, name="ysb")
                    nc.vector.tensor_add(ysb[:], yab[j][:, icp, :], bob_t[:])
                    nc.sync.dma_start(y_d[128 * ic:128 * (ic + 1), :], ysb[:])

                op_full(0); op_fin(4); op_full(1); op_fin(5)
                op_full(2); op_fin(6); op_full(3); op_fin(7)

    return nc


def _get_program():
    if "nc" not in _prog_cache:
        nc = _build_program()
        if not nc.is_finalized():
            nc.finalize()
        _prog_cache["nc"] = nc
    return _prog_cache["nc"]


def _packT(m):
    """[R, C] -> [128, R//128, C] so one DMA fills a [128, R//128 * C] tile."""
    r, c = m.shape
    return np.ascontiguousarray(
        m.reshape(r // 128, 128, c).transpose(1, 0, 2))


def _prep_core(b, x, mask, wq, bq, wk, bk, wv, bv, wo, bo):
    import ml_dtypes

    b16 = ml_dtypes.bfloat16
    f = np.float32
    xb = np.ascontiguousarray(x[b], dtype=f)                # [N, D]
    idx = np.nonzero(mask[b])[0]
    nv = int(idx.size)
    assert 1 <= nv <= NKV, f"batch {b}: {nv} valid keys, NKV={NKV}"
    xk = np.zeros((NKV, D), f)
    xk[:nv] = xb[idx]
    pos = np.arange(128)[:, None] + 128 * np.arange(KC)[None, :]
    expb = np.where(pos < nv, 0.0, PAD_BIAS).astype(f)      # [128, KC]
    tbl = np.concatenate(
        [np.ascontiguousarray(bq, f).reshape(DC, 128).T, expb], axis=1)
    bob = (bo.astype(f) + bv.astype(f) @ wo.astype(f)).reshape(D)
    return {
        "xT": _packT(np.ascontiguousarray(xb.T)).astype(b16),
        "xkT": _packT(np.ascontiguousarray(xk.T)).astype(b16),
        "wq": _packT(np.ascontiguousarray(wq, f)).astype(b16),
        "wk": _packT(np.ascontiguousarray(wk, f)).astype(b16),
        "wv": _packT(np.ascontiguousarray(wv, f)).astype(b16),
        "wo": _packT(np.ascontiguousarray(wo, f)).astype(b16),
        "tbl": np.ascontiguousarray(tbl),
        "bob": np.ascontiguousarray(np.broadcast_to(bob, (128, D))),
    }


def _run(inputs):
    import os

    os.environ["BASS_NEVER_TRACE"] = "1"
    from concourse.bass_utils import run_bass_kernel_spmd

    nc = _get_program()
    in_maps = [_prep_core(b, **inputs) for b in range(B)]
    res = run_bass_kernel_spmd(nc, in_maps, core_ids=list(range(B)),
                               trace=False)
    out = np.stack([res.results[b]["y"] for b in range(B)], axis=0)
    return out.astype(np.float32), res


def kernel(**inputs) -> np.ndarray:
    out, _ = _run(inputs)
    return out


# revision 39
# speedup vs baseline: 1.4026x; 1.4026x over previous
"""Multi-head attention (B=8, N=1024, D=512, H=8) on 8 TRN2 NeuronCores.

Sharding: pure batch-parallel - core i computes batch i end-to-end, no
collectives. Host-side prep per batch: gather valid keys (mask) into a
contiguous buffer padded to NKV=640, pre-transpose x, convert streams to
bf16, and pack every stream so it loads with ONE dma_start (issue on the
shared HWDGE costs ~630ns per instruction and a single queue streams at
~150-200GB/s, so the loads are spread across the SP and ACT HWDGE queues
plus the gpsimd SWDGE queue, with the x streams split in halves).

Device pipeline (bf16 matmuls, f32 PSUM accumulation):
  k^T/q^T/v projections -> per head pair, scores s^T[k,q] land in one
  [128,1024] PSUM tile (head A cols 0:512 on PE row tile 0, head B cols
  512:1024 on row tile 64, overlapping on disjoint PE rows) -> one exp on
  ACT per query-half with the key-padding mask folded into the activation
  bias -> attn@v with an augmented ones-column producing the softmax
  denominator in row 64; the four (head, half) accumulators of a pair
  ring through a 4-buffer PSUM pool so the next pair's AV starts while
  this pair's normalize chains (denominator copy, fast reciprocal, gpsimd
  partition-broadcast, multiply fused with the PSUM->SBUF copy) drain ->
  out-projection. The tail is restructured so the last pair's chain
  latency is covered: out-proj columns for ic 4..7 pre-accumulate their
  dp0-2 terms into retired score-pool PSUM slots, ic 0..3 run from the
  retiring AV slots, and only 4 matmuls + bias + store remain after the
  final chain. y is stored bf16 and upcast on the host.

Math shortcuts: bk is dropped (constant-in-key terms cancel in softmax);
bv is folded into the output bias on the host (bob' = bo + bv @ wo since
normalized attention rows sum to 1). fp8 attn@v was tried and rejected:
v quantization alone puts the max-abs tail at 2.5e-2 (tol 2e-2) and exp
overflows e4m3. Interleaving the projections into the attention phase
was tried and rejected: the PE row rate stays ~0.83ns/row regardless, so
it only moves work between phases, while shrinking the AV PSUM pool to
make room serializes the normalize chains into the pipeline.
"""

import sys

import numpy as np

sys.path.insert(0, "/opt/trn_rl_repo")

B, N, D, H = 8, 1024, 512, 8
HD = D // H            # 64
SCALE = HD ** -0.5     # 0.125
NKV = 640              # padded valid-key count (5 chunks of 128)
KC = NKV // 128        # 5
DC = D // 128          # 4
VW = HD + 2            # 66: aug head stride, 4B-aligned for bf16 weights
PAD_BIAS = -30000.0    # exp(PAD_BIAS + s*SCALE) == 0.0 exactly

_prog_cache = {}


def _build_program():
    import concourse.bacc as bacc
    import concourse.tile as tile
    from concourse import mybir

    dt = mybir.dt
    f32 = dt.float32
    bf16 = dt.bfloat16
    AF = mybir.ActivationFunctionType

    nc = bacc.Bacc("TRN2", target_bir_lowering=False, debug=False)

    xT_d = nc.dram_tensor("xT", [128, DC, N], bf16, kind="ExternalInput").ap()
    xkT_d = nc.dram_tensor("xkT", [128, DC, NKV], bf16,
                           kind="ExternalInput").ap()
    wq_d = nc.dram_tensor("wq", [128, DC, D], bf16, kind="ExternalInput").ap()
    wk_d = nc.dram_tensor("wk", [128, DC, D], bf16, kind="ExternalInput").ap()
    wv_d = nc.dram_tensor("wv", [128, DC, D], bf16, kind="ExternalInput").ap()
    wo_d = nc.dram_tensor("wo", [128, DC, D], bf16, kind="ExternalInput").ap()
    tbl_d = nc.dram_tensor("tbl", [128, DC + KC], f32,
                           kind="ExternalInput").ap()
    bob_d = nc.dram_tensor("bob", [128, D], f32, kind="ExternalInput").ap()
    y_d = nc.dram_tensor("y", [N, D], bf16, kind="ExternalOutput").ap()

    with tile.TileContext(nc) as tc, \
         nc.allow_low_precision(reason="bf16 matmul streams, f32 accum"):
        with tc.tile_pool(name="const", bufs=1) as cpool:
            wk_t = cpool.tile([128, DC, D], bf16, name="wk_t")
            wq_t = cpool.tile([128, DC, D], bf16, name="wq_t")
            wv_t = cpool.tile([128, DC, D], bf16, name="wv_t")
            wo_t = cpool.tile([128, DC, D], bf16, name="wo_t")
            xkT_t = cpool.tile([128, DC, NKV], bf16, name="xkT_t")
            xT_t = cpool.tile([128, DC, N], bf16, name="xT_t")
            kT_t = cpool.tile([128, DC, NKV], bf16, name="kT_t")
            qT_t = cpool.tile([128, DC, N], bf16, name="qT_t")
            vaug_t = [cpool.tile([128, H, VW], bf16, name=f"vaug_t{c}")
                      for c in range(KC)]
            aoT_t = cpool.tile([128, DC, N], bf16, name="aoT_t")
            tbl_t = cpool.tile([128, DC + KC], f32, name="tbl_t")
            bob_t = cpool.tile([128, D], f32, name="bob_t")

            # kproj's inputs split across both HWDGE queues so all four
            # halves land concurrently (~150GB/s per queue); qproj's xT
            # second half rides the SWDGE queue which is otherwise idle
            # until wv is needed.
            nc.sync.dma_start(xkT_t[:, 0:2, :], xkT_d[:, 0:2, :])
            nc.sync.dma_start(wk_t[:, 0:2, :], wk_d[:, 0:2, :])
            nc.sync.dma_start(xT_t[:, 0:2, :], xT_d[:, 0:2, :])
            nc.scalar.dma_start(tbl_t[:], tbl_d[:])
            nc.scalar.dma_start(xkT_t[:, 2:4, :], xkT_d[:, 2:4, :])
            nc.scalar.dma_start(wk_t[:, 2:4, :], wk_d[:, 2:4, :])
            nc.scalar.dma_start(wq_t[:], wq_d[:])
            nc.gpsimd.dma_start(xT_t[:, 2:4, :], xT_d[:, 2:4, :])
            nc.gpsimd.dma_start(wv_t[:], wv_d[:])
            nc.gpsimd.dma_start(bob_t[:], bob_d[:])
            nc.gpsimd.dma_start(wo_t[:], wo_d[:])

            # ones column of the augmented v (denominator trick), plus a
            # dummy exp to pull the ACT table load off the first real exp
            warm_t = cpool.tile([1, 1], f32, name="warm_t")
            nc.vector.memset(warm_t[:], 0.0)
            nc.scalar.activation(warm_t[:], warm_t[:],
                                 mybir.ActivationFunctionType.Exp)
            for c in range(KC):
                nc.vector.memset(vaug_t[c][:, :, HD:HD + 1], 1.0)

            # ---- Phase 1a: k projection (no bias: cancels in softmax) ----
            with tc.tile_pool(name="kpp", bufs=3, space="PSUM") as kpp:
                for dp in range(DC):
                    ps = kpp.tile([128, NKV], f32, name="kps")
                    for dc in range(DC):
                        lhs = wk_t[:, dc, 128 * dp:128 * (dp + 1)]
                        nc.tensor.matmul(
                            ps[:, 0:512], lhs, xkT_t[:, dc, 0:512],
                            start=(dc == 0), stop=(dc == DC - 1),
                        )
                        nc.tensor.matmul(
                            ps[:, 512:NKV], lhs, xkT_t[:, dc, 512:NKV],
                            start=(dc == 0), stop=(dc == DC - 1),
                        )
                    nc.vector.tensor_scalar_add(kT_t[:, dp, :], ps[:], 0.0)

            # ---- Phase 1b: q projection ----
            with tc.tile_pool(name="qpp", bufs=3, space="PSUM") as qpp:
                for dp in range(DC):
                    ps = qpp.tile([128, N], f32, name="qps")
                    for dc in range(DC):
                        lhs = wq_t[:, dc, 128 * dp:128 * (dp + 1)]
                        for hf in range(2):
                            nc.tensor.matmul(
                                ps[:, 512 * hf:512 * (hf + 1)],
                                lhs,
                                xT_t[:, dc, 512 * hf:512 * (hf + 1)],
                                start=(dc == 0), stop=(dc == DC - 1),
                            )
                    nc.vector.tensor_scalar_add(qT_t[:, dp, :], ps[:],
                                                tbl_t[:, dp:dp + 1])

            # ---- Phase 1c: v projection (no bias: folded into bob') ----
            with tc.tile_pool(name="vpp", bufs=2, space="PSUM") as vpp:
                for c in range(KC):
                    ps = vpp.tile([128, H, HD], f32, name="vps")
                    for dc in range(DC):
                        nc.tensor.matmul(
                            ps[:], xkT_t[:, dc, 128 * c:128 * (c + 1)],
                            wv_t[:, dc, :],
                            start=(dc == 0), stop=(dc == DC - 1),
                        )
                    nc.vector.tensor_scalar_add(vaug_t[c][:, :, 0:HD],
                                                ps[:], 0.0)

            # ---- Phase 2: attention on head pairs + restructured tail ----
            with tc.tile_pool(name="scp", bufs=2, space="PSUM") as scp, \
                 tc.tile_pool(name="oap", bufs=4, space="PSUM") as oap, \
                 tc.tile_pool(name="pp", bufs=10) as pp, \
                 tc.tile_pool(name="rcp", bufs=6) as rcp, \
                 tc.tile_pool(name="rbp", bufs=4) as rbp, \
                 tc.tile_pool(name="ysp", bufs=4) as ysp:
                # Software-pipelined across head-pair boundaries: scores
                # run two chunks ahead of attn@v GLOBALLY, so the ACT exp
                # stream never drains while a new pair's pipeline refills.
                oa_t = {}   # (dp, hf, hi) -> [65, 512] PSUM accumulator
                p_t = {}    # (dp, c, hf)  -> [128, N] bf16 exp tile

                def sc_unit(dp, c):
                    for hf in range(2):
                        sc = scp.tile([128, N], f32, name="sc", tag="sc")
                        for hi in range(2):
                            row = HD * hi
                            nc.tensor.matmul(
                                sc[:, 512 * hi:512 * (hi + 1)],
                                kT_t[row:row + HD, dp,
                                     128 * c:128 * (c + 1)],
                                qT_t[row:row + HD, dp,
                                     512 * hf:512 * (hf + 1)],
                                start=True, stop=True,
                            )
                        p = pp.tile([128, N], bf16, name="p")
                        nc.scalar.activation(
                            p[:], sc[:], AF.Exp,
                            bias=tbl_t[:, DC + c:DC + c + 1], scale=SCALE,
                        )
                        p_t[(dp, c, hf)] = p

                def av_unit(dp, c):
                    for hf in range(2):
                        for hi in range(2):
                            if c == 0:
                                oa_t[(dp, hf, hi)] = oap.tile(
                                    [HD + 1, 512], f32, name="oa", tag="oa")
                            nc.tensor.matmul(
                                oa_t[(dp, hf, hi)][:],
                                vaug_t[c][:, 2 * dp + hi, 0:HD + 1],
                                p_t[(dp, c, hf)][:, 512 * hi:512 * (hi + 1)],
                                start=(c == 0), stop=(c == KC - 1),
                            )

                def chains(dp, db_on_act=False):
                    for hf in range(2):
                        for hi in range(2):
                            oa = oa_t[(dp, hf, hi)]
                            row = HD * hi
                            # custom DVE ops read garbage from PSUM on HW:
                            # stage the denominator row through SBUF first
                            # (on ACT for the last pair - it idles there
                            # while DVE still runs the normalize muls)
                            db = rcp.tile([1, 512], f32, name="db")
                            if db_on_act:
                                nc.scalar.copy(db[:], oa[HD:HD + 1, :])
                            else:
                                nc.vector.tensor_scalar_add(
                                    db[:], oa[HD:HD + 1, :], 0.0)
                            rc = rcp.tile([1, 512], f32, name="rc")
                            nc.vector.reciprocal_approx_fast(rc[:], db[:])
                            rbs = rbp.tile([HD, 512], f32, name="rbs")
                            nc.gpsimd.partition_broadcast(rbs[:], rc[:])
                            nc.vector.tensor_mul(
                                aoT_t[row:row + HD, dp,
                                      512 * hf:512 * (hf + 1)],
                                oa[0:HD, :], rbs[:])

                seq = [(dp, c) for dp in range(DC) for c in range(KC)]
                for i, (dp, c) in enumerate(seq):
                    sc_unit(dp, c)
                    if i >= 3:
                        adp, ac = seq[i - 3]
                        av_unit(adp, ac)
                        if ac == KC - 1:
                            chains(adp)
                for j in (len(seq) - 3, len(seq) - 2, len(seq) - 1):
                    adp, ac = seq[j]
                    av_unit(adp, ac)
                    if ac == KC - 1:
                        chains(adp, db_on_act=True)

                # ---- restructured out-projection tail ----
                # ic 4..7: pre-accumulate dp0-2 into retired score slots
                # while the dp3 chains drain (their aoT needs only chains
                # of dp0-2, done long ago, plus dp3 which lands last).
                yab = []
                for j in range(2):
                    t = scp.tile([128, 2, 512], f32, name="yab", tag="sc")
                    for icp in range(2):
                        ic = 4 + 2 * j + icp
                        for dp in range(3):
                            nc.tensor.matmul(
                                t[:, icp, :],
                                aoT_t[:, dp, 128 * ic:128 * (ic + 1)],
                                wo_t[:, dp, :], start=(dp == 0), stop=False)
                    yab.append(t)
                # ic 0..3 (full accumulation from the retiring AV slots)
                # interleaved with the ic 4..7 dp3-term finishers so no
                # single consumer chain serializes the stream.
                def op_full(ic):
                    yps = oap.tile([128, 512], f32, name="yps", tag="oa")
                    for dp in range(DC):
                        nc.tensor.matmul(
                            yps[:], aoT_t[:, dp, 128 * ic:128 * (ic + 1)],
                            wo_t[:, dp, :],
                            start=(dp == 0), stop=(dp == DC - 1),
                        )
                    ysb = ysp.tile([128, D], bf16, name="ysb")
                    nc.vector.tensor_add(ysb[:], yps[:], bob_t[:])
                    nc.sync.dma_start(y_d[128 * ic:128 * (ic + 1), :], ysb[:])

                def op_fin(ic):
                    j, icp = divmod(ic - 4, 2)
                    nc.tensor.matmul(
                        yab[j][:, icp, :],
                        aoT_t[:, 3, 128 * ic:128 * (ic + 1)],
                        wo_t[:, 3, :], start=False, stop=True)
                    ysb = ysp.tile([128, D], bf16, name="ysb")
                    nc.vector.tensor_add(ysb[:], yab[j][:, icp, :], bob_t[:])
                    nc.sync.dma_start(y_d[128 * ic:128 * (ic + 1), :], ysb[:])

                op_full(0); op_fin(4); op_full(1); op_fin(5)
                op_full(2); op_fin(6); op_full(3); op_fin(7)

    return nc


def _get_program():
    if "nc" not in _prog_cache:
        nc = _build_program()
        if not nc.is_finalized():
            nc.finalize()
        _prog_cache["nc"] = nc
    return _prog_cache["nc"]


def _packT(m):
    """[R, C] -> [128, R//128, C] so one DMA fills a [128, R//128 * C] tile."""
    r, c = m.shape
    return np.ascontiguousarray(
        m.reshape(r // 128, 128, c).transpose(1, 0, 2))


def _prep_core(b, x, mask, wq, bq, wk, bk, wv, bv, wo, bo):
    import ml_dtypes

    b16 = ml_dtypes.bfloat16
    f = np.float32
    xb = np.ascontiguousarray(x[b], dtype=f)                # [N, D]
    idx = np.nonzero(mask[b])[0]
    nv = int(idx.size)
    assert 1 <= nv <= NKV, f"batch {b}: {nv} valid keys, NKV={NKV}"
    xk = np.zeros((NKV, D), f)
    xk[:nv] = xb[idx]
    pos = np.arange(128)[:, None] + 128 * np.arange(KC)[None, :]
    expb = np.where(pos < nv, 0.0, PAD_BIAS).astype(f)      # [128, KC]
    tbl = np.concatenate(
        [np.ascontiguousarray(bq, f).reshape(DC, 128).T, expb], axis=1)
    bob = (bo.astype(f) + bv.astype(f) @ wo.astype(f)).reshape(D)
    return {
        "xT": _packT(np.ascontiguousarray(xb.T)).astype(b16),
        "xkT": _packT(np.ascontiguousarray(xk.T)).astype(b16),
        "wq": _packT(np.ascontiguousarray(wq, f)).astype(b16),
        "wk": _packT(np.ascontiguousarray(wk, f)).astype(b16),
        "wv": _packT(np.ascontiguousarray(wv, f)).astype(b16),
        "wo": _packT(np.ascontiguousarray(wo, f)).astype(b16),
        "tbl": np.ascontiguousarray(tbl),
        "bob": np.ascontiguousarray(np.broadcast_to(bob, (128, D))),
    }


def _run(inputs):
    import os

    os.environ["BASS_NEVER_TRACE"] = "1"
    from concourse.bass_utils import run_bass_kernel_spmd

    nc = _get_program()
    in_maps = [_prep_core(b, **inputs) for b in range(B)]
    res = run_bass_kernel_spmd(nc, in_maps, core_ids=list(range(B)),
                               trace=False)
    out = np.stack([res.results[b]["y"] for b in range(B)], axis=0)
    return out.astype(np.float32), res


def kernel(**inputs) -> np.ndarray:
    out, _ = _run(inputs)
    return out
